# revision 20
# baseline (speedup 1.0000x reference)
"""Trainium2 Bass kernel for nn_BinarizeLayer (chain Viterbi binarization).

Algorithm (bar space)
---------------------
The reference is a 2-state Viterbi DP over an 8.4M-node chain.  With
d_i = a0_i - a1_i the forward pass collapses to

    d_i = e_i + clamp(d_{i-1}, -lam, lam),      e_i = 2*p_i - 1.

Conjugating by prefix sums and rescaling by 1/(2*lam) ("bar space"):
host sends xbar_i = p_i/lam - 0.5/lam, device computes

    sbar  = cumsum(xbar)                 (radix-2 tensor_tensor_scan)
    w_k   = max(min(w_{k-1}, sbar_{k-1}), sbar_{k-1} - 1)   (TTScan min/max)
    label = reversed fold y' = [w_k - y < sbar_{k+1} - 1]   (TTScan sub/is_lt)

The radix-2 sum scan reads host-deinterleaved even/odd streams through
the two TTScan operand slots, computing two chain elements per scan
element (scan elements cost 2 cycles; this restores ~1 cycle/element).
Even-position sums are recovered off the DVE critical engine: the PE
computes I @ sbar_odd_shifted + I @ xbar_even into PSUM (identity
matmuls are bit-exact in fp32), and the Act engine evicts them into the
interleaved sbar tile and also produces sbar - 1.

Sharding: 8 cores x 128 partition rows of 8192 payload elements, 64-halo
on both sides (clamp recurrences forget their initial state within ~64
steps).  Global chain ends are padded with p=0.5 (xbar = 0.0 exactly);
the final-label boundary condition is injected by overwriting the last
sbar-1 column with sbar - 0.5 (sentinel: y_init=0 -> [d_last > 0]).
"""

import numpy as np

import concourse.bass as bass
import concourse.mybir as mybir
from concourse import tile
from concourse import bass_utils

LAM = 0.75
N = 8388608
NCORES = 8
P = 128          # partitions
W = 64           # halo / warm-up width
D = 8192         # payload elements per partition row
R = D + 2 * W    # row length incl. halos
RX = R + P       # input row incl. trailing identity matrix columns

# forward blocks: small first blocks to start compute early
_FW = [256, 256, 512, 1024, 2048, 2048, 2048 + 2 * W]
FBLK = []
_c = 0
for _w in _FW:
    FBLK.append((_c, _w))
    _c += _w
# backscan payload blocks (cs, payload width); small last block cuts the tail
BBLK = [(W, 2048), (W + 2048, 2048), (W + 4096, 2048), (W + 6144, 1792),
        (W + 7936, 256)]
MMC = 512        # matmul output chunk (one PSUM bank of fp32)


def _build():
    f32 = mybir.dt.float32
    i8 = mybir.dt.int8
    Alu = mybir.AluOpType
    Copy = mybir.ActivationFunctionType.Copy

    nc = bass.Bass()
    x = nc.dram_tensor("x", [P, RX], f32, kind="ExternalInput")
    y = nc.dram_tensor("y", [P, D], i8, kind="ExternalOutput")

    with tile.TileContext(nc) as tc:
        with tc.tile_pool(name="big", bufs=1) as big, \
             tc.tile_pool(name="ps", bufs=2, space="PSUM") as psp:
            XP = big.tile([P, R], f32)        # prescaled input (deinterleaved)
            ID = big.tile([P, P], f32)        # identity weights
            SODD = big.tile([P, R // 2 + 1], f32)  # packed odd-position sums
            SB = big.tile([P, R + 1], f32)    # SB[j+1] = sbar incl thru j
            SBM = big.tile([P, R + 1], f32)   # SB - 1 (sentinel at col R)
            WT = big.tile([P, R], f32)        # clamp walk
            LB = big.tile([P, R], i8)         # labels

            nc.vector.memset(SB[:, 0:1], 0.0)
            nc.vector.memset(SODD[:, 0:1], 0.0)
            nc.vector.memset(SBM[:, 0:1], -1.0)
            # prewarm the Act function table while the first DMA is in flight
            nc.scalar.activation(SBM[:, 0:1], SBM[:, 0:1], Copy, bias=0.0)
            # identity weights via the Act engine's DMA queue so the input
            # blocks on the Sync queue are not delayed
            nc.scalar.dma_start(ID[:], x[:, R:RX])

            for (c0, bw) in FBLK:
                # host deposits this block deinterleaved: [evens | odds]
                h = bw // 2
                g = c0 // 2
                nc.sync.dma_start(XP[:, c0:c0 + bw], x[:, c0:c0 + bw])
                # radix-2 sum scan -> packed odd-position sums (contiguous
                # in AND out; the scan chains scan-to-scan through SODD)
                init = 0.0 if c0 == 0 else SODD[:, g:g + 1]
                nc.vector.tensor_tensor_scan(
                    SODD[:, g + 1:g + 1 + h],
                    XP[:, c0:c0 + h],
                    XP[:, c0 + h:c0 + bw],
                    init, Alu.add, Alu.add)
                # even chain positions via PE: sbar_even = sbar_odd_prev
                # + xbar_even, accumulated bit-exactly in PSUM
                PS = psp.tile([P, h], f32)
                for m0 in range(0, h, MMC):
                    mw = min(MMC, h - m0)
                    nc.tensor.matmul(
                        PS[:, m0:m0 + mw], ID[:],
                        SODD[:, g + m0:g + m0 + mw],
                        start=True, stop=False)
                    nc.tensor.matmul(
                        PS[:, m0:m0 + mw], ID[:],
                        XP[:, c0 + m0:c0 + m0 + mw],
                        start=False, stop=True)
                # Act interleaves both parities into SB (strided writes run
                # at full rate on Act) and produces SBM = SB - 1
                nc.scalar.activation(SB[:, c0 + 1:c0 + 1 + bw:2], PS[:],
                                     Copy, bias=0.0)
                nc.scalar.activation(SB[:, c0 + 2:c0 + 1 + bw:2],
                                     SODD[:, g + 1:g + 1 + h],
                                     Copy, bias=0.0)
                nc.scalar.activation(SBM[:, c0 + 1:c0 + 1 + bw],
                                     SB[:, c0 + 1:c0 + 1 + bw],
                                     Copy, bias=-1.0)
                # clamp walk: w' = max(min(w, sbar_excl), sbar_excl - 1)
                winit = -0.5 if c0 == 0 else WT[:, c0 - 1:c0]
                nc.vector.tensor_tensor_scan(
                    WT[:, c0:c0 + bw], SB[:, c0:c0 + bw],
                    SBM[:, c0:c0 + bw], winit, Alu.min, Alu.max)

            # boundary sentinel: SBM[R] = SB[R] - 0.5 so that with y=0 the
            # first reversed step computes [d_last > 0]
            nc.scalar.activation(SBM[:, R:R + 1], SB[:, R:R + 1],
                                 Copy, bias=-0.5)

            # backscans: y' = is_lt(w - y, sbm_incl), reversed, W warm-up
            for (cs, bell) in BBLK:
                wd = bell + W
                nc.vector.tensor_tensor_scan(
                    LB[:, cs:cs + wd][:, ::-1],
                    WT[:, cs:cs + wd][:, ::-1],
                    SBM[:, cs + 1:cs + 1 + wd][:, ::-1],
                    0.0, Alu.subtract, Alu.is_lt)
                nc.sync.dma_start(y[:, cs - W:cs - W + bell],
                                  LB[:, cs:cs + bell])
    return nc


def _legalize_waits(nc, limit=1):
    """Split instructions carrying more than `limit` sem-waits.

    This walrus build rejects instructions whose sync_info has more wait
    commands than the ISA encoding allows (Tile can accumulate several).
    Excess waits move onto NoOps prepended on the same engine, which
    preserves per-engine ordering semantics.
    """
    import concourse.mybir as mybir
    for fn in nc.m.functions:
        for blk in fn.blocks:
            insts = blk.instructions
            i = 0
            while i < len(insts):
                inst = insts[i]
                si = getattr(inst, "sync_info", None)
                if si is not None and si.on_wait and len(si.on_wait) > limit:
                    waits = list(si.on_wait)
                    inst.sync_info = mybir.SyncInfo(
                        on_wait=waits[-limit:], on_update=list(si.on_update))
                    pending = waits[:-limit]
                    for j in range(0, len(pending), limit):
                        nop = mybir.InstNoOp(
                            name=nc.get_next_instruction_name(),
                            sync_info=mybir.SyncInfo(
                                on_wait=pending[j:j + limit], on_update=[]),
                            bass_nofuse=True,
                            engine=inst.engine,
                        )
                        insts.insert(i, nop)
                        i += 1
                i += 1
    return nc


_nc_cache = None


def _get_nc():
    global _nc_cache
    if _nc_cache is None:
        _nc_cache = _legalize_waits(_build())
    return _nc_cache


LAST_RESULT = None


def kernel(inputs: np.ndarray, _trace: bool = False) -> np.ndarray:
    global LAST_RESULT
    f = np.float32
    p = np.ascontiguousarray(inputs, dtype=f)
    assert p.shape == (N,)
    pad = np.full(W, 0.5, f)
    pp = np.concatenate([pad, p, pad])
    # host prescale into bar space: xbar = p/lam - 0.5/lam (pad -> 0.0)
    xb = (pp * f(1.0 / LAM) + f(-0.5 / LAM)).astype(f)
    nrows = N // D
    X = np.lib.stride_tricks.as_strided(xb, (nrows, R), (D * 4, 4))
    # deinterleave each forward block ([evens | odds]) and append the
    # identity weight matrix in the trailing P columns
    XD = np.empty((nrows, RX), f)
    for (c0, bw) in FBLK:
        h = bw // 2
        XD[:, c0:c0 + h] = X[:, c0:c0 + bw:2]
        XD[:, c0 + h:c0 + bw] = X[:, c0 + 1:c0 + bw:2]
    idm = np.eye(P, dtype=f)
    for k in range(NCORES):
        XD[k * P:(k + 1) * P, R:RX] = idm
    in_maps = [{"x": np.ascontiguousarray(XD[k * P:(k + 1) * P])}
               for k in range(NCORES)]
    res = bass_utils.run_bass_kernel_spmd(_get_nc(), in_maps,
                                          core_ids=list(range(NCORES)),
                                          trace=_trace)
    LAST_RESULT = res
    lab = np.concatenate([np.asarray(res.results[k]["y"]).reshape(-1)
                          for k in range(NCORES)])
    return lab.astype(np.int32)
